# revision 11
# baseline (speedup 1.0000x reference)
"""Trainium2 Bass kernel for nn_DepthNet (plane-sweep MVS depth regression).

Strategy (self-contained; shapes hardcoded for B=2, C=32, H=96, W=128, D=48,
V=2 source views, 8 NeuronCores):

- Sharding: core = b*4 + hs handles batch b and target-row slab
  rows [24*hs, 24*hs+24). All 48 depths local -> no collectives.
- Key structural fact (checked exactly on host): for every pixel the source
  sample points across all 48 depths stay within a 2-wide x-window and a
  2-row y-window (epipolar span << 1 px). One 1KB DMA-gather per
  (pixel, view) fetches a 4x2 source window; bilinear-with-zero-pad is then
  exactly  sum_{i,j} relu(1-|vy-i|) relu(1-|u-j|) * maskcell_ij * Q_ij.
- The per-channel warp never gets materialized: with hat weights affine in 6
  monomials per view, the variance+1x1x1-conv cost collapses to a per-pixel
  polynomial:  cost = (2/9) [g_rr + g11 + g22 - g_r1 - g_r2 - g12] + b_reg,
  each g a quadratic form with per-pixel Gram coefficients (w-weighted dots
  over C) that are depth-independent.  Device evaluates ~90 polynomial terms
  on [128 pixel-partitions, 24x48 free] tiles, then softmax over D, depth
  regression and photometric confidence.
"""
import numpy as np

H, W, D, C, V, B = 96, 128, 48, 32, 2, 2
SLAB = H // 4            # 24 rows per core
NPIX = SLAB * W          # 3072 pixels per core
PAD = 8                  # table padding rows (each 128 px) above/below
TROWS = H * W + 2 * PAD * W   # 14336 table rows
ECOLS = 8 * C            # 256 f32 per table row (4 cols x 2 rows x 32 ch)

_cache = {}


def _combine_proj(proj):
    ext = np.array(proj[..., 0, :, :], np.float32)
    K = np.array(proj[..., 1, :3, :3], np.float32)
    out = ext.copy()
    out[..., :3, :4] = np.einsum('...ij,...jk->...ik', K, ext[..., :3, :4])
    return out


def _wrap16(idx_flat):
    n = idx_flat.shape[0]
    blk = idx_flat.reshape(n // 16, 16).T.astype(np.int16)
    t = np.zeros((128, n // 16), np.int16)
    for a in range(8):
        t[16 * a:16 * (a + 1)] = blk
    return t


def _host_prep(ref_feature, src_features, ref_proj, src_projs, depth_values,
               w_reg, b_reg):
    f32 = np.float32
    ref_p = _combine_proj(np.asarray(ref_proj, f32))
    src_p = _combine_proj(np.asarray(src_projs, f32))
    depth = np.asarray(depth_values, f32)
    yy, xx = np.meshgrid(np.arange(H, dtype=f32), np.arange(W, dtype=f32),
                         indexing='ij')
    xyz = np.stack([xx.ravel(), yy.ravel(), np.ones(H * W, f32)])

    # quad tables: per (v, b) a [TROWS, ECOLS] f32 array, row k_tab holds the
    # 4x2 source window starting at flat position k_tab - PAD*W.
    tabs = {}
    geo = {}
    for v in range(V):
        for b in range(B):
            proj = (src_p[v, b] @ np.linalg.inv(ref_p[b].astype(np.float64))
                    ).astype(f32)
            rot, trans = proj[:3, :3], proj[:3, 3]
            rx = (rot @ xyz).astype(f32)
            Xd = rx[0][None] * depth[b][:, None] + trans[0]
            Yd = rx[1][None] * depth[b][:, None] + trans[1]
            Zd = rx[2][None] * depth[b][:, None] + trans[2]
            px = (Xd / Zd).astype(f32)
            py = (Yd / Zd).astype(f32)
            kx = np.floor(px.min(0)).astype(np.int64)
            ky = np.floor(py[0]).astype(np.int64)
            assert (px >= kx).all() and (px < kx + 2).all(), \
                "DepthNet fast path: x-window condition violated"
            assert (py >= ky - 1).all() and (py < ky + 2).all(), \
                "DepthNet fast path: y-window condition violated"
            geo[(v, b)] = dict(rx=rx, trans=trans, kx=kx, ky=ky)

            fea = np.asarray(src_features[v, b], f32).reshape(C, H * W)
            fcl = np.zeros((TROWS + 3 + W + 3, C), f32)
            fcl[PAD * W:PAD * W + H * W] = fea.T
            offs = [0, 1, 2, 3, W, W + 1, W + 2, W + 3]
            tab = np.concatenate([fcl[o:o + TROWS] for o in offs], axis=1)
            tabs[(v, b)] = np.ascontiguousarray(tab)

    # per-core shipped tensors
    w_pos = (2.0 / 9.0) * np.asarray(w_reg, f32)
    breg = np.asarray(b_reg, f32)[0]
    in_maps = []
    for core in range(8):
        b, hs = core // 4, core % 4
        p0 = hs * SLAB * W                      # first flat pixel of slab
        sl = slice(p0, p0 + NPIX)
        m = {}
        for v in range(V):
            g = geo[(v, b)]
            kx, ky = g['kx'][sl], g['ky'][sl]
            ktab = (ky + PAD) * W + kx
            ktab = np.clip(ktab, 0, TROWS - 1)   # fully-masked px may be OOB
            m[f'tab{v}'] = tabs[(v, b)]
            m[f'idx{v}'] = _wrap16(ktab)
            # [128 part=w, 24 col=h] layouts
            m[f'kx{v}'] = kx.reshape(SLAB, W).T.astype(f32).copy()
            m[f'ky{v}'] = ky.reshape(SLAB, W).T.astype(f32).copy()
            rxs = g['rx'][:, sl].reshape(3, SLAB, W)
            m[f'rx{v}'] = np.ascontiguousarray(
                rxs.transpose(2, 0, 1).reshape(W, 3 * SLAB))
            # cell masks [128, 6*24]: order (i*3+j, col)
            mk = np.zeros((W, 6, SLAB), f32)
            kxg = kx.reshape(SLAB, W)
            kyg = ky.reshape(SLAB, W)
            for i in (0, 1):
                for j in (0, 1, 2):
                    valid = ((kxg + j >= 0) & (kxg + j <= W - 1)
                             & (kyg + i >= 0) & (kyg + i <= H - 1))
                    mk[:, i * 3 + j, :] = valid.T.astype(f32)
            m[f'mk{v}'] = mk.reshape(W, 6 * SLAB)
        ts = np.zeros(8, f32)
        ts[0:3] = geo[(0, b)]['trans']
        ts[3:6] = geo[(1, b)]['trans']
        ts[6] = breg
        m['tsc'] = np.broadcast_to(ts, (128, 8)).copy()
        m['dep'] = np.broadcast_to(depth[b], (128, D)).copy()
        m['iota48'] = np.broadcast_to(np.arange(D, dtype=f32), (128, D)).copy()
        m['wpos'] = np.broadcast_to(w_pos, (128, C)).copy()
        m['ident'] = np.eye(128, dtype=f32)
        refb = np.asarray(ref_feature[b], f32).reshape(C, H, W)[:, hs * SLAB:(hs + 1) * SLAB]
        # ref_cl [128 part=w, 24 col, 32 c]
        m['refcl'] = np.ascontiguousarray(
            refb.transpose(2, 1, 0).reshape(W, SLAB * C))
        in_maps.append(m)
    return in_maps


def _ap(t, pat, offset=0):
    """Build an AP on tile t with explicit free pattern (list of (step,num)),
    partition dim = full 128."""
    import concourse.bass as bass
    base = t[:]
    ap = [base.ap[0]] + [list(p) for p in pat]
    return bass.AP(base.tensor, base.offset + offset, ap)


def _build_nc():
    import concourse.bacc as bacc
    import concourse.bass as bass
    import concourse.tile as tile
    from concourse import mybir
    dt = mybir.dt
    AF = mybir.ActivationFunctionType
    AL = mybir.AluOpType
    AX = mybir.AxisListType

    nc = bacc.Bacc("TRN2", target_bir_lowering=False, debug=False,
                   num_devices=8)
    f32 = dt.float32
    for cval in (-1.0,):
        _ct = nc.alloc_sbuf_tensor(f"const-f32-{cval}", [128, 1], f32)
        nc.gpsimd.memset(_ct.ap(), cval)
        nc.const_aps.aps[(f32, cval)] = _ct.ap()
    nc.all_engine_barrier()
    din = {}
    for v in range(V):
        din[f'tab{v}'] = nc.dram_tensor(f'tab{v}', [TROWS, ECOLS], f32, kind="ExternalInput").ap()
        din[f'idx{v}'] = nc.dram_tensor(f'idx{v}', [128, NPIX // 16], dt.int16, kind="ExternalInput").ap()
        for nm in ('kx', 'ky'):
            din[f'{nm}{v}'] = nc.dram_tensor(f'{nm}{v}', [128, SLAB], f32, kind="ExternalInput").ap()
        din[f'rx{v}'] = nc.dram_tensor(f'rx{v}', [128, 3 * SLAB], f32, kind="ExternalInput").ap()
        din[f'mk{v}'] = nc.dram_tensor(f'mk{v}', [128, 6 * SLAB], f32, kind="ExternalInput").ap()
    for nm, sh in [('tsc', [128, 8]), ('dep', [128, D]), ('iota48', [128, D]),
                   ('wpos', [128, C]), ('ident', [128, 128]),
                   ('refcl', [128, SLAB * C])]:
        din[nm] = nc.dram_tensor(nm, sh, f32, kind="ExternalInput").ap()
    d_prob = nc.dram_tensor('prob_slab', [D, SLAB * W], f32, kind="ExternalOutput").ap()
    d_dep = nc.dram_tensor('depth_slab', [SLAB, W], f32, kind="ExternalOutput").ap()
    d_conf = nc.dram_tensor('conf_slab', [SLAB, W], f32, kind="ExternalOutput").ap()

    F = SLAB * D         # 1152
    FC = SLAB * C        # 768

    with tile.TileContext(nc) as tc:
        with tc.tile_pool(name="inp", bufs=1) as inp, \
             tc.tile_pool(name="gath", bufs=1) as gath, \
             tc.tile_pool(name="wst", bufs=1) as wst, \
             tc.tile_pool(name="mono", bufs=1) as mono, \
             tc.tile_pool(name="kc", bufs=1) as kc, \
             tc.tile_pool(name="accp", bufs=2) as accp, \
             tc.tile_pool(name="one", bufs=1) as one, \
             tc.tile_pool(name="tmp", bufs=2) as tmp, \
             tc.tile_pool(name="out", bufs=1) as outp, \
             tc.tile_pool(name="ps", bufs=2, space="PSUM") as ps:

            # ---- load inputs
            sb = {}
            for nm, ap_ in din.items():
                if nm.startswith('tab'):
                    continue
                shp = list(ap_.shape)
                t = inp.tile(shp, dt.int16 if nm.startswith('idx') else f32, tag=nm)
                nc.sync.dma_start(t[:], ap_[:])
                sb[nm] = t

            # ---- per view: gather (shared slot), geometry/monomials, W-stack
            M = []    # monomial tiles [128, 6, F]
            WS = []   # W-vector stacks [128, 6, SLAB, C]
            for v in range(V):
                q = gath.tile([128, SLAB, ECOLS], f32, tag="q")
                for ch in range(3):
                    nc.gpsimd.dma_gather(
                        q[:, ch * 8:(ch + 1) * 8, :],
                        din[f'tab{v}'][:],
                        sb[f'idx{v}'][:, ch * 64:(ch + 1) * 64],
                        num_idxs=1024, num_idxs_reg=1024, elem_size=ECOLS)

                rxt = sb[f'rx{v}']
                dep = sb['dep']
                tsc = sb['tsc']

                def madd_axis(k, tag):
                    t = tmp.tile([128, F], f32, tag="ta")
                    nc.vector.tensor_tensor(
                        t[:],
                        _ap(rxt, [(1, SLAB), (0, D)], offset=k * SLAB),
                        _ap(dep, [(0, SLAB), (1, D)]),
                        AL.mult)
                    t2 = tmp.tile([128, F], f32, tag=tag)
                    nc.vector.tensor_tensor(
                        t2[:], t[:], _ap(tsc, [(0, SLAB), (0, D)], offset=3 * v + k),
                        AL.add)
                    return t2

                Zt = madd_axis(2, "tb")
                iz = tmp.tile([128, F], f32, tag="tc")
                nc.vector.reciprocal(iz[:], Zt[:])
                Xt = madd_axis(0, "tb")
                px = tmp.tile([128, F], f32, tag="td")
                nc.vector.tensor_tensor(px[:], Xt[:], iz[:], AL.mult)
                Yt = madd_axis(1, "tb")
                py = tmp.tile([128, F], f32, tag="tc")
                nc.vector.tensor_tensor(py[:], Yt[:], iz[:], AL.mult)
                u = tmp.tile([128, F], f32, tag="ta")
                nc.vector.tensor_tensor(
                    u[:], px[:], _ap(sb[f'kx{v}'], [(1, SLAB), (0, D)]),
                    AL.subtract)
                vy = tmp.tile([128, F], f32, tag="tb")
                nc.vector.tensor_tensor(
                    vy[:], py[:], _ap(sb[f'ky{v}'], [(1, SLAB), (0, D)]),
                    AL.subtract)
                mt = mono.tile([128, 6, F], f32, tag=f"m{v}")
                a0 = tmp.tile([128, F], f32, tag="td")
                nc.scalar.activation(a0[:], u[:], AF.Relu, bias=1.0, scale=-1.0)
                t = tmp.tile([128, F], f32, tag="tc")
                nc.scalar.activation(t[:], u[:], AF.Abs, bias=-1.0, scale=1.0)
                a1 = tmp.tile([128, F], f32, tag="ta")
                nc.scalar.activation(a1[:], t[:], AF.Relu, bias=1.0, scale=-1.0)
                t2b = tmp.tile([128, F], f32, tag="tc")
                nc.scalar.activation(t2b[:], vy[:], AF.Abs, bias=0.0, scale=1.0)
                r0 = mt[:, 0, :]
                nc.scalar.activation(r0, t2b[:], AF.Relu, bias=1.0, scale=-1.0)
                t3 = tmp.tile([128, F], f32, tag="tc")
                nc.scalar.activation(t3[:], vy[:], AF.Abs, bias=-1.0, scale=1.0)
                r1 = mt[:, 3, :]
                nc.scalar.activation(r1, t3[:], AF.Relu, bias=1.0, scale=-1.0)
                nc.vector.tensor_tensor(mt[:, 1, :], mt[:, 0, :], a0[:], AL.mult)
                nc.vector.tensor_tensor(mt[:, 2, :], mt[:, 0, :], a1[:], AL.mult)
                nc.vector.tensor_tensor(mt[:, 4, :], mt[:, 3, :], a0[:], AL.mult)
                nc.vector.tensor_tensor(mt[:, 5, :], mt[:, 3, :], a1[:], AL.mult)
                M.append(mt)

                mk = sb[f'mk{v}']
                ws = wst.tile([128, 6, SLAB, C], f32, tag=f"ws{v}")
                def cell_ap(i, j, q=q):
                    return _ap(q, [(ECOLS, SLAB), (1, C)], offset=(i * 4 + j) * C)
                def mask_ap(i, j, mk=mk):
                    return _ap(mk, [(1, SLAB), (0, C)], offset=(i * 3 + j) * SLAB)
                nc.vector.tensor_tensor(ws[:, 0, :, :], cell_ap(0, 2), mask_ap(0, 2), AL.mult)
                nc.vector.tensor_tensor(ws[:, 3, :, :], cell_ap(1, 2), mask_ap(1, 2), AL.mult)
                for i, j, dst in ((0, 0, 1), (0, 1, 2), (1, 0, 4), (1, 1, 5)):
                    tq = tmp.tile([128, SLAB, C], f32, tag="cell")
                    nc.vector.tensor_tensor(tq[:, :, :], cell_ap(i, j), mask_ap(i, j), AL.mult)
                    nc.vector.tensor_tensor(
                        ws[:, dst, :, :], tq[:, :, :],
                        ws[:, 0 if i == 0 else 3, :, :], AL.subtract)
                WS.append(ws)

            # ---- Gram dots
            wpos = sb['wpos']
            refw = one.tile([128, SLAB, C], f32, tag="refw")
            refcl = sb['refcl']
            nc.vector.tensor_tensor(
                refw[:, :, :], _ap(refcl, [(1, FC)]),
                _ap(wpos, [(0, SLAB), (1, C)]), AL.mult)

            K11 = kc.tile([128, 6, 6, SLAB], f32, tag="k11")
            K22 = kc.tile([128, 6, 6, SLAB], f32, tag="k22")
            K12 = kc.tile([128, 6, 6, SLAB], f32, tag="k12")
            Kr1 = kc.tile([128, 6, SLAB], f32, tag="kr1")
            Kr2 = kc.tile([128, 6, SLAB], f32, tag="kr2")
            K0 = kc.tile([128, SLAB], f32, tag="k0")
            for a in range(6):
                eng = nc.vector if a % 2 == 0 else nc.gpsimd
                wa1 = one.tile([128, SLAB, C], f32, tag="wa1")
                eng.tensor_tensor(
                    wa1[:, :, :], WS[0][:, a, :, :],
                    _ap(wpos, [(0, SLAB), (1, C)]), AL.mult)
                wa2 = one.tile([128, SLAB, C], f32, tag="wa2")
                eng.tensor_tensor(
                    wa2[:, :, :], WS[1][:, a, :, :],
                    _ap(wpos, [(0, SLAB), (1, C)]), AL.mult)
                pr = gath.tile([128, 6, SLAB, C], f32, tag="q")
                eng.tensor_tensor(
                    pr[:, :, :, :], _ap(wa1, [(0, 6), (1, SLAB * C)]),
                    WS[0][:, :, :, :], AL.mult)
                nc.vector.tensor_reduce(K11[:, a, :, :], pr[:, :, :, :], AX.X, AL.add)
                pr2 = gath.tile([128, 6, SLAB, C], f32, tag="q")
                eng.tensor_tensor(
                    pr2[:, :, :, :], _ap(wa2, [(0, 6), (1, SLAB * C)]),
                    WS[1][:, :, :, :], AL.mult)
                nc.vector.tensor_reduce(K22[:, a, :, :], pr2[:, :, :, :], AX.X, AL.add)
                pr3 = gath.tile([128, 6, SLAB, C], f32, tag="q")
                eng.tensor_tensor(
                    pr3[:, :, :, :], _ap(wa1, [(0, 6), (1, SLAB * C)]),
                    WS[1][:, :, :, :], AL.mult)
                nc.vector.tensor_reduce(K12[:, a, :, :], pr3[:, :, :, :], AX.X, AL.add)
            prr = gath.tile([128, 6, SLAB, C], f32, tag="q")
            nc.vector.tensor_tensor(
                prr[:, :, :, :], _ap(refw, [(0, 6), (1, SLAB * C)]),
                WS[0][:, :, :, :], AL.mult)
            nc.vector.tensor_reduce(Kr1[:, :, :], prr[:, :, :, :], AX.X, AL.add)
            prr2 = gath.tile([128, 6, SLAB, C], f32, tag="q")
            nc.gpsimd.tensor_tensor(
                prr2[:, :, :, :], _ap(refw, [(0, 6), (1, SLAB * C)]),
                WS[1][:, :, :, :], AL.mult)
            nc.vector.tensor_reduce(Kr2[:, :, :], prr2[:, :, :, :], AX.X, AL.add)
            prr3 = one.tile([128, SLAB, C], f32, tag="prr3")
            nc.vector.tensor_tensor(
                prr3[:, :, :], refw[:, :, :], _ap(refcl, [(1, FC)]), AL.mult)
            nc.vector.tensor_reduce(K0[:, :], prr3[:, :, :], AX.X, AL.add)

            # ---- polynomial evaluation
            # acc = K0 + breg  (broadcast over d), then += terms
            acc = accp.tile([128, F], f32, tag="acc")
            nc.vector.tensor_tensor(
                acc[:], _ap(K0, [(1, SLAB), (0, D)]),
                _ap(sb['tsc'], [(0, SLAB), (0, D)], offset=6), AL.add)

            def kap(K, a, b):
                return _ap(K, [(1, SLAB), (0, D)], offset=(a * 6 + b) * SLAB)

            def kap1(K, a):
                return _ap(K, [(1, SLAB), (0, D)], offset=a * SLAB)

            m1, m2 = M
            terms = []
            for a in range(6):
                for b in range(a, 6):
                    fac = 1.0 if a == b else 2.0
                    terms.append((m1[:, a, :], m1[:, b, :], kap(K11, a, b), fac, AL.add))
                    terms.append((m2[:, a, :], m2[:, b, :], kap(K22, a, b), fac, AL.add))
            for a in range(6):
                for b in range(6):
                    terms.append((m1[:, a, :], m2[:, b, :], kap(K12, a, b), 1.0, AL.subtract))
            for a in range(6):
                terms.append((None, m1[:, a, :], kap1(Kr1, a), 1.0, AL.subtract))
                terms.append((None, m2[:, a, :], kap1(Kr2, a), 1.0, AL.subtract))

            n_pool = 0
            for i, (ma, mb, kapx, fac, op) in enumerate(terms):
                # Pool cannot run scalar_tensor_tensor (TensorScalarPtr);
                # route factor-2 terms to DVE, alternate the rest.
                if fac == 2.0:
                    eng = nc.vector
                else:
                    n_pool += 1
                    eng = nc.gpsimd if n_pool % 2 == 0 else nc.vector
                if ma is not None:
                    prod = tmp.tile([128, F], f32, tag="ta" if eng is nc.vector else "tc")
                    eng.tensor_tensor(prod[:], ma, mb, AL.mult)
                    src_ap = prod[:]
                else:
                    src_ap = mb
                kprod = tmp.tile([128, F], f32, tag="tb" if eng is nc.vector else "td")
                if fac == 2.0:
                    eng.scalar_tensor_tensor(kprod[:], src_ap, 2.0, kapx, AL.mult, AL.mult)
                else:
                    eng.tensor_tensor(kprod[:], src_ap, kapx, AL.mult)
                acc2 = accp.tile([128, F], f32, tag="acc")
                eng.tensor_tensor(acc2[:], acc[:], kprod[:], op)
                acc = acc2

            # ---- softmax over d, depth regression, confidence
            cost3 = _ap(acc, [(D, SLAB), (1, D)])  # view [128, SLAB, D]
            cmax = outp.tile([128, SLAB], f32, tag="cmax")
            nc.vector.tensor_reduce(cmax[:], cost3, AX.X, AL.max)
            es = accp.tile([128, F], f32, tag="acc")
            nc.vector.tensor_tensor(
                _ap(es, [(D, SLAB), (1, D)]), cost3,
                _ap(cmax, [(1, SLAB), (0, D)]), AL.subtract)
            ex = accp.tile([128, F], f32, tag="acc")
            nc.scalar.activation(ex[:], es[:], AF.Exp)
            ssum = outp.tile([128, SLAB], f32, tag="ssum")
            nc.vector.tensor_reduce(ssum[:], _ap(ex, [(D, SLAB), (1, D)]), AX.X, AL.add)
            rsum = outp.tile([128, SLAB], f32, tag="rsum")
            nc.vector.reciprocal(rsum[:], ssum[:])
            prob = accp.tile([128, F], f32, tag="prob")
            nc.vector.tensor_tensor(
                _ap(prob, [(D, SLAB), (1, D)]),
                _ap(ex, [(D, SLAB), (1, D)]),
                _ap(rsum, [(1, SLAB), (0, D)]), AL.mult)

            dep = sb['dep']
            pd = tmp.tile([128, F], f32, tag="ta")
            nc.vector.tensor_tensor(
                _ap(pd, [(D, SLAB), (1, D)]),
                _ap(prob, [(D, SLAB), (1, D)]),
                _ap(dep, [(0, SLAB), (1, D)]), AL.mult)
            depth_t = outp.tile([128, SLAB], f32, tag="dept")
            nc.vector.tensor_reduce(depth_t[:], _ap(pd, [(D, SLAB), (1, D)]), AX.X, AL.add)

            pi = tmp.tile([128, F], f32, tag="tb")
            nc.vector.tensor_tensor(
                _ap(pi, [(D, SLAB), (1, D)]),
                _ap(prob, [(D, SLAB), (1, D)]),
                _ap(sb['iota48'], [(0, SLAB), (1, D)]), AL.mult)
            didx = outp.tile([128, SLAB], f32, tag="didx")
            nc.vector.tensor_reduce(didx[:], _ap(pi, [(D, SLAB), (1, D)]), AX.X, AL.add)
            # floor+clip didx
            ii = outp.tile([128, SLAB], mybir_int32(), tag="dii")
            nc.vector.tensor_copy(ii[:], didx[:])
            fl = outp.tile([128, SLAB], f32, tag="dfl")
            nc.vector.tensor_copy(fl[:], ii[:])
            gt = outp.tile([128, SLAB], f32, tag="dgt")
            nc.vector.tensor_tensor(gt[:], fl[:], didx[:], AL.is_gt)
            fl2 = outp.tile([128, SLAB], f32, tag="dfl2")
            nc.vector.tensor_tensor(fl2[:], fl[:], gt[:], AL.subtract)
            fl3 = outp.tile([128, SLAB], f32, tag="dfl3")
            nc.vector.tensor_scalar(fl3[:], fl2[:], 0.0, float(D - 1), AL.max, AL.min)

            # sum2 = prob + shift(prob); conf = sum(sum2 * (iota==didx))
            sum2 = tmp.tile([128, F], f32, tag="ta")
            nc.vector.tensor_tensor(
                _ap(sum2, [(D, SLAB), (1, D - 1)]),
                _ap(prob, [(D, SLAB), (1, D - 1)]),
                _ap(prob, [(D, SLAB), (1, D - 1)], offset=1), AL.add)
            nc.vector.tensor_copy(
                _ap(sum2, [(D, SLAB), (1, 1)], offset=D - 1),
                _ap(prob, [(D, SLAB), (1, 1)], offset=D - 1))
            msk = tmp.tile([128, F], f32, tag="tb")
            nc.vector.tensor_tensor(
                _ap(msk, [(D, SLAB), (1, D)]),
                _ap(sb['iota48'], [(0, SLAB), (1, D)]),
                _ap(fl3, [(1, SLAB), (0, D)]), AL.is_equal)
            cm = tmp.tile([128, F], f32, tag="tc")
            nc.vector.tensor_tensor(cm[:], sum2[:], msk[:], AL.mult)
            conf_t = outp.tile([128, SLAB], f32, tag="conft")
            nc.vector.tensor_reduce(conf_t[:], _ap(cm, [(D, SLAB), (1, D)]), AX.X, AL.add)

            # ---- output transposes + DMA
            ident = sb['ident']
            # prob: [128 w, (SLAB, D)] -> per col h: [128, D] -> T -> [D, 128]
            pstage = mono.tile([D, SLAB, 128], f32, tag="m0")
            for col in range(SLAB):
                pt = ps.tile([D, 128], f32, tag="pt")
                nc.tensor.transpose(
                    pt[:], _ap(prob, [(1, D)], offset=col * D), ident[:])
                nc.scalar.copy(pstage[:, col, :], pt[:])
            nc.sync.dma_start(d_prob[:], pstage[:, :, :])

            ostage = mono.tile([SLAB, 2, 128], f32, tag="m1")
            for (srct, oi) in ((depth_t, 0), (conf_t, 1)):
                ot = ps.tile([SLAB, 128], f32, tag="ot")
                nc.tensor.transpose(ot[:], srct[:], ident[:])
                nc.scalar.copy(ostage[:, oi, :], ot[:])
            nc.sync.dma_start(d_dep[:], ostage[:, 0, :])
            nc.sync.dma_start(d_conf[:], ostage[:, 1, :])

    nc.compile()
    return nc


def mybir_int32():
    from concourse import mybir
    return mybir.dt.int32


class _Runner:
    def __init__(self, nc, n_cores=8):
        import jax
        import jax.numpy  # noqa
        from jax.sharding import Mesh, PartitionSpec
        from jax.experimental.shard_map import shard_map
        from concourse import mybir
        from concourse.bass2jax import (_bass_exec_p, install_neuronx_cc_hook,
                                        partition_id_tensor)
        install_neuronx_cc_hook()
        self.jax = jax
        self.nc = nc
        self.n_cores = n_cores
        part_name = nc.partition_id_tensor.name if nc.partition_id_tensor else None
        in_names, out_names, out_avals, zero_outs = [], [], [], []
        for alloc in nc.m.functions[0].allocations:
            if not isinstance(alloc, mybir.MemoryLocationSet):
                continue
            name = alloc.memorylocations[0].name
            if alloc.kind == "ExternalInput":
                if name != part_name:
                    in_names.append(name)
            elif alloc.kind == "ExternalOutput":
                out_names.append(name)
                shape = tuple(alloc.tensor_shape)
                dtype = mybir.dt.np(alloc.dtype)
                out_avals.append(jax.core.ShapedArray(shape, dtype))
                zero_outs.append(np.zeros(shape, dtype))
        self.in_names, self.out_names = in_names, out_names
        self.out_avals, self.zero_outs = out_avals, zero_outs
        n_params, n_outs = len(in_names), len(out_avals)
        all_in = list(in_names) + list(out_names)
        if part_name is not None:
            all_in = all_in + [part_name]

        def _body(*args):
            operands = list(args)
            if part_name is not None:
                operands.append(partition_id_tensor())
            outs = _bass_exec_p.bind(
                *operands, out_avals=tuple(out_avals), in_names=tuple(all_in),
                out_names=tuple(out_names), lowering_input_output_aliases=(),
                sim_require_finite=True, sim_require_nnan=True, nc=nc)
            return tuple(outs)

        devices = jax.devices()[:n_cores]
        self.mesh = Mesh(np.asarray(devices), ("core",))
        in_specs = (PartitionSpec("core"),) * (n_params + n_outs)
        out_specs = (PartitionSpec("core"),) * n_outs
        self.fn = jax.jit(
            shard_map(_body, mesh=self.mesh, in_specs=in_specs,
                      out_specs=out_specs, check_rep=False), keep_unused=True)

    def prepare(self, in_maps):
        from jax.sharding import PartitionSpec
        n = self.n_cores
        per_core = [[np.asarray(m[name]) for name in self.in_names] for m in in_maps]
        concat = [np.concatenate([per_core[c][i] for c in range(n)], axis=0)
                  for i in range(len(self.in_names))]
        concat += [np.zeros((n * z.shape[0], *z.shape[1:]), z.dtype)
                   for z in self.zero_outs]
        sharding = self.jax.sharding.NamedSharding(self.mesh, PartitionSpec("core"))
        return [self.jax.device_put(a, sharding) for a in concat]

    def run(self, args):
        outs = self.fn(*args)
        self.jax.block_until_ready(outs)
        return outs

    def results(self, outs):
        n = self.n_cores
        return [
            {name: np.asarray(outs[i]).reshape(n, *self.out_avals[i].shape)[c]
             for i, name in enumerate(self.out_names)}
            for c in range(n)]


def _get_runtime():
    if 'rt' not in _cache:
        nc = _build_nc()
        _cache['rt'] = _Runner(nc)
    return _cache['rt']


def kernel(ref_feature, src_features, ref_proj, src_projs, depth_values,
           w_reg, b_reg, num_depth):
    in_maps = _host_prep(ref_feature, src_features, ref_proj, src_projs,
                         depth_values, w_reg, b_reg)
    rt = _get_runtime()
    res = rt.results(rt.run(rt.prepare(in_maps)))
    f32 = np.float32
    depth_out = np.zeros((B, H, W), f32)
    conf_out = np.zeros((B, H, W), f32)
    prob_out = np.zeros((B, D, H, W), f32)
    for core in range(8):
        b, hs = core // 4, core % 4
        rows = slice(hs * SLAB, (hs + 1) * SLAB)
        depth_out[b, rows] = res[core]['depth_slab']
        conf_out[b, rows] = res[core]['conf_slab']
        prob_out[b, :, rows] = res[core]['prob_slab'].reshape(D, SLAB, W)
    return depth_out, conf_out, prob_out
